# revision 18
# baseline (speedup 1.0000x reference)
"""Center-update (scatter-add) kernel for Trainium2, 8 NeuronCores.

Math: given features [B, D], labels [B], centers [N, D]:
    diff        = (ALPHA - 1) * (centers[labels] - features)
    new_centers = centers.at[labels].add(diff)
which reduces per center row n to
    new_centers[n] = centers[n] * (1 - 0.1*count[n]) + 0.1 * featsum[n]
with count = histogram(labels), featsum = segment-sum of features by label.

Sharding: centers are sharded along N across the 8 cores (12500 rows each).
Feature rows are routed all-to-all by label bucket (host computes the
bucket/sort metadata; each core receives the feature rows whose labels land
in its bucket, in original row order).  On device, each 128-center tile
gathers its feature rows via indirect DMA into a [128 rows, 257] tile
(column 256 preset to 1.0 to produce counts), multiplies with a one-hot
matrix (built on-device from iota + per-row slot ids; value 0.1) on the
tensor engine to produce per-center 0.1*featsum and 0.1*count in PSUM, then
combines with the centers tile and writes the output shard contiguously.
"""
import sys
import types
import numpy as np

if '/opt/trn_rl_repo' not in sys.path:
    sys.path.insert(0, '/opt/trn_rl_repo')

import concourse.bass as bass
import concourse.mybir as mybir
import concourse.tile as tile
from concourse import bass_utils
from concourse import library_config

ALPHA = 0.9
SCALE = 1.0 - ALPHA  # 0.1
IOTA_MAT = np.tile(np.arange(128, dtype=np.float32), (128, 1))
N_CORES = 8
B, D, N = 65536, 256, 100000
NS = N // N_CORES  # centers per core
P = 128

F32 = mybir.dt.float32
I32 = mybir.dt.int32
I16 = mybir.dt.int16


def _patch_drain_and_barrier():
    """This walrus build encodes at most one sync-wait on the CTRL-format
    Drain instruction; split the Tile exit drain's waits across single-wait
    sync nops."""
    if getattr(tile.TileContext, '_drain_patched', False):
        return

    def _drain_and_barrier(self, tick_clock, wait_clock):
        from concourse.tile import ScopedClock
        nc = self.nc
        drain_inst = nc.sync.drain()
        wait_clock.add_sem_waits(
            drain_inst.ins, ScopedClock({None: tick_clock.global_clock})
        )
        si = drain_inst.ins.sync_info
        waits = list(si.on_wait) if si and si.on_wait else []
        if len(waits) > 1:
            si.on_wait.clear()
            si.on_wait.append(waits[0])
            for w in waits[1:]:
                nop = nc.sync.nop()
                nsi = nop.ins.sync_info
                if nsi is None:
                    nop.ins.sync_info = mybir.SyncInfo(on_wait=[w], on_update=[])
                else:
                    nsi.on_wait.append(w)
        nc.all_engine_barrier()
        popped = nc._tile_sem_poison_stack.pop()
        assert popped is self._sem_poison
        nc.clear_and_free_semaphores(list(self.sems.allocated().values()))
        nc.all_engine_barrier()

    tile.TileContext._drain_and_barrier = _drain_and_barrier
    tile.TileContext._drain_patched = True


_patch_drain_and_barrier()


def _split_multi_waits(nc):
    """This walrus build encodes only ONE sync-wait per instruction (any
    format).  Hoist every extra wait onto an InstNoOp inserted immediately
    before the instruction on the same engine (per-engine program order
    within a block makes the nops' waits complete first)."""
    for f in nc.m.functions:
        for bb in f.blocks:
            new_insts = []
            for inst in bb.instructions:
                si = inst.sync_info
                waits = list(si.on_wait) if si and si.on_wait else []
                if len(waits) > 1:
                    si.on_wait.clear()
                    for w in waits[:-1]:
                        nop = mybir.InstNoOp(
                            name=nc.get_next_instruction_name(), ins=[], outs=[]
                        )
                        nop.engine = inst.engine
                        nop.sync_info = mybir.SyncInfo(on_wait=[w], on_update=[])
                        nc.register_instruction(nop, overwrite=True)
                        new_insts.append(nop)
                    si.on_wait.append(waits[-1])
                new_insts.append(inst)
            bb.instructions[:] = new_insts


def build_routing(labels, n_cores=N_CORES, ns=NS, p=P):
    """Host-side sharding metadata.  Returns per-core
    (shard_rows, gidx[P, C] int32, slots[P, C] f32) plus cols_per_tile.

    shard_rows: global feature-row indices owned by core k (original order).
    gidx[:, c]: local (within-shard) feature rows feeding column c.
    slots[:, c]: slot (center - tile_base) per row, -1.0 for padding.
    Column -> center-tile map (cols_per_tile) is identical across cores.
    """
    labels = np.asarray(labels).astype(np.int64).ravel()
    t_tiles = (ns + p - 1) // p
    shard_rows = []
    loc_sorted = []   # per core: bucket-local labels, sorted
    lidx_sorted = []  # per core: local shard index per sorted row
    for k in range(n_cores):
        lo = k * ns
        rows = np.nonzero((labels >= lo) & (labels < lo + ns))[0]
        loc = labels[rows] - lo
        order = np.argsort(loc, kind='stable')
        shard_rows.append(rows)
        loc_sorted.append(loc[order])
        lidx_sorted.append(order.astype(np.int64))

    # rows per (core, tile) -> shared column structure
    counts = np.zeros((n_cores, t_tiles), dtype=np.int64)
    for k in range(n_cores):
        tl = loc_sorted[k] // p
        cnt = np.bincount(tl, minlength=t_tiles)
        counts[k] = cnt[:t_tiles]
    cols_per_tile = np.maximum(1, -(-counts.max(axis=0) // p)).astype(int)
    c_total = int(cols_per_tile.sum())

    gidx_all, slots_all = [], []
    for k in range(n_cores):
        gidx = np.zeros((p, c_total), dtype=np.int64)
        slots = np.full((p, c_total), -1.0, dtype=np.float32)
        tl = loc_sorted[k] // p
        starts = np.searchsorted(tl, np.arange(t_tiles))
        ends = np.searchsorted(tl, np.arange(t_tiles), side='right')
        col = 0
        for t in range(t_tiles):
            sel = slice(starts[t], ends[t])
            lidx = lidx_sorted[k][sel]
            slot = (loc_sorted[k][sel] - t * p).astype(np.float32)
            r = len(lidx)
            for j in range(cols_per_tile[t]):
                a, b = j * p, min((j + 1) * p, r)
                if a < b:
                    gidx[: b - a, col] = lidx[a:b]
                    slots[: b - a, col] = slot[a:b]
                col += 1
        # dma_gather index layout: flat position i = col*128 + p (the gather
        # writes dst[i%128, i//128]); int16, wrapped idx[i] -> [i%16, i//16],
        # replicated across all 8 groups of 16 partitions.
        flat = gidx.T.ravel()  # position i = c*128 + p
        assert flat.max(initial=0) < 32768
        n_i = flat.size  # = c_total * 128, multiple of 16
        wrapped = flat.reshape(n_i // 16, 16).T.astype(np.int16)  # [16, n/16]
        gidx_all.append(np.tile(wrapped, (8, 1)))  # [128, n/16]
        slots_all.append(slots)
    return shard_rows, gidx_all, slots_all, cols_per_tile


def build_program(c_total, cols_per_tile, fpad, ns=NS, d=D, chunk_tiles=7):
    """Build the (SPMD-shared) Bass program."""
    p = P
    t_tiles = len(cols_per_tile)
    nidx_total = c_total * p
    nc = bass.Bass()
    feats = nc.declare_dram_parameter('feats', [fpad, d], F32, isOutput=False)
    centers = nc.declare_dram_parameter('centers', [ns, d], F32, isOutput=False)
    gidx_d = nc.declare_dram_parameter('gidx', [p, nidx_total // 16], I16, isOutput=False)
    slots_d = nc.declare_dram_parameter('slots', [p, c_total], F32, isOutput=False)
    iotam_d = nc.declare_dram_parameter('iotam', [p, p], F32, isOutput=False)
    out = nc.declare_dram_parameter('out', [ns, d], F32, isOutput=True)

    W = d + 1  # psum width: 256 featsum cols + 1 count col

    with tile.TileContext(nc) as tc:
        with (
            tc.tile_pool(name='const', bufs=1) as cpool,
            tc.tile_pool(name='gather', bufs=3) as gpool,
            tc.tile_pool(name='cent', bufs=4) as centpool,
            tc.tile_pool(name='outp', bufs=4) as opool,
            tc.tile_pool(name='oh', bufs=4) as ohpool,
            tc.tile_pool(name='scale', bufs=4) as spool,
            tc.tile_pool(name='psum', bufs=4, space='PSUM') as pspool,
            tc.tile_pool(name='psumc', bufs=3, space='PSUM') as pscpool,
        ):
            nc.gpsimd.load_library(library_config.mlp)
            iota_f = cpool.tile([p, p], F32)
            nc.sync.dma_start(out=iota_f[:], in_=iotam_d[:])
            ones = cpool.tile([p, 1], F32)
            nc.vector.memset(ones[:], 1.0)
            gidx_sb = cpool.tile([p, nidx_total // 16], I16)
            slots_sb = cpool.tile([p, c_total], F32)
            nc.sync.dma_start(out=gidx_sb[:], in_=gidx_d[:])
            nc.sync.dma_start(out=slots_sb[:], in_=slots_d[:])

            col0 = 0
            for cs in range(0, t_tiles, chunk_tiles):
                chunk = range(cs, min(cs + chunk_tiles, t_tiles))
                ncols = int(sum(cols_per_tile[t] for t in chunk))
                nidx = ncols * p
                gbuf = gpool.tile([p, ncols * d], F32, tag='gbuf')
                g3 = gbuf[:].rearrange('p (c w) -> p c w', w=d)
                nc.gpsimd.dma_gather(
                    out_ap=g3[:, :, :],
                    in_ap=feats[:],
                    idxs_ap=gidx_sb[:, col0 * 8:(col0 + ncols) * 8],
                    num_idxs=nidx,
                    num_idxs_reg=nidx,
                    elem_size=d,
                )
                cj = 0
                for t in chunk:
                    pt = min(p, ns - t * p)
                    cent = centpool.tile([p, d], F32, tag='cent')
                    nc.sync.dma_start(out=cent[:pt, :], in_=centers[t * p:t * p + pt, :])
                    ps = pspool.tile([p, d], F32, tag='ps')
                    psc = pscpool.tile([p, 1], F32, tag='psc')
                    nct = int(cols_per_tile[t])
                    for j in range(nct):
                        oh = ohpool.tile([p, p], F32, tag='oh')
                        nc.vector.tensor_scalar(
                            oh[:], iota_f[:],
                            slots_sb[:, col0 + cj:col0 + cj + 1], SCALE,
                            op0=mybir.AluOpType.is_equal,
                            op1=mybir.AluOpType.mult,
                        )
                        nc.tensor.matmul(
                            ps[:], lhsT=oh[:], rhs=gbuf[:, cj * d:(cj + 1) * d],
                            start=(j == 0), stop=(j == nct - 1),
                        )
                        nc.tensor.matmul(
                            psc[:], lhsT=oh[:], rhs=ones[:, :],
                            start=(j == 0), stop=(j == nct - 1),
                        )
                        cj += 1
                    # scale_vec = 1 - 0.1*count  (psc holds 0.1*count)
                    scale = spool.tile([p, 1], F32, tag='scale')
                    nc.scalar.activation(
                        scale[:], psc[:],
                        mybir.ActivationFunctionType.Identity,
                        bias=1.0, scale=-1.0,
                    )
                    ot = opool.tile([p, d], F32, tag='ot')
                    nc.vector.tensor_scalar(
                        ot[:pt, :], cent[:pt, :], scale[:pt, :], None,
                        op0=mybir.AluOpType.mult,
                    )
                    nc.vector.tensor_tensor(
                        ot[:pt, :], ot[:pt, :], ps[:pt, :],
                        op=mybir.AluOpType.add,
                    )
                    nc.sync.dma_start(out=out[t * p:t * p + pt, :], in_=ot[:pt, :])
                col0 += ncols
    _split_multi_waits(nc)
    # encode .instr bytes for extended-ISA instructions (dma_gather,
    # library reload) — bacc normally does this; raw Bass+Tile must not skip
    # it or walrus fails with "ISA wrong length"
    mybir.codegen_inst_isa_subclasses(nc)
    return nc


_PROGRAM_CACHE = {}

# test-harness knobs: when TRACE is set, pass trace=True through to
# run_bass_kernel_spmd and stash the BassKernelResults in LAST_RESULTS.
TRACE = False
TRACE_TMPDIR = None
LAST_RESULTS = None


def _get_program(c_total, cols_key, fpad):
    key = (c_total, cols_key, fpad)
    if key not in _PROGRAM_CACHE:
        _PROGRAM_CACHE[key] = build_program(
            c_total, np.asarray(cols_key, dtype=int), fpad
        )
    return _PROGRAM_CACHE[key]


def kernel(features, labels, centers):
    features = np.ascontiguousarray(np.asarray(features), dtype=np.float32)
    centers_np = np.ascontiguousarray(np.asarray(centers), dtype=np.float32)
    labels_np = np.asarray(labels)

    shard_rows, gidx_all, slots_all, cols_per_tile = build_routing(labels_np)
    c_total = int(cols_per_tile.sum())
    fpad = max(1, max(len(r) for r in shard_rows))

    nc = _get_program(c_total, tuple(int(x) for x in cols_per_tile), fpad)

    in_maps = []
    for k in range(N_CORES):
        fshard = np.zeros((fpad, D), dtype=np.float32)
        rows = shard_rows[k]
        fshard[: len(rows)] = features[rows]
        in_maps.append({
            'feats': fshard,
            'centers': centers_np[k * NS:(k + 1) * NS],
            'gidx': gidx_all[k],
            'slots': slots_all[k],
            'iotam': IOTA_MAT,
        })

    kwargs = {}
    if TRACE:
        kwargs['trace'] = True
        if TRACE_TMPDIR:
            kwargs['tmpdir'] = TRACE_TMPDIR
    res = bass_utils.run_bass_kernel_spmd(
        nc, in_maps, core_ids=list(range(N_CORES)), **kwargs
    )
    global LAST_RESULTS
    LAST_RESULTS = res
    out = np.concatenate([res.results[k]['out'] for k in range(N_CORES)], axis=0)
    return out


# revision 19
# speedup vs baseline: 1.4742x; 1.4742x over previous
"""Center-update (scatter-add) kernel for Trainium2, 8 NeuronCores.

Math: given features [B, D], labels [B], centers [N, D]:
    diff        = (ALPHA - 1) * (centers[labels] - features)
    new_centers = centers.at[labels].add(diff)
which reduces per center row n to
    new_centers[n] = centers[n] * (1 - 0.1*count[n]) + 0.1 * featsum[n]
with count = histogram(labels), featsum = segment-sum of features by label.

Sharding: centers are sharded along N across the 8 cores (12500 rows each).
Feature rows are routed all-to-all by label bucket (host computes the
bucket/sort metadata; each core receives the feature rows whose labels land
in its bucket, in original row order).  On device, each 128-center tile
gathers its feature rows via indirect DMA into a [128 rows, 257] tile
(column 256 preset to 1.0 to produce counts), multiplies with a one-hot
matrix (built on-device from iota + per-row slot ids; value 0.1) on the
tensor engine to produce per-center 0.1*featsum and 0.1*count in PSUM, then
combines with the centers tile and writes the output shard contiguously.
"""
import sys
import types
import numpy as np

if '/opt/trn_rl_repo' not in sys.path:
    sys.path.insert(0, '/opt/trn_rl_repo')

import concourse.bass as bass
import concourse.mybir as mybir
import concourse.tile as tile
from concourse import bass_utils
from concourse import library_config

ALPHA = 0.9
SCALE = 1.0 - ALPHA  # 0.1
IOTA_MAT = np.tile(np.arange(128, dtype=np.float32), (128, 1))
N_CORES = 8
B, D, N = 65536, 256, 100000
NS = N // N_CORES  # centers per core
P = 128

F32 = mybir.dt.float32
I32 = mybir.dt.int32
I16 = mybir.dt.int16


def _patch_drain_and_barrier():
    """This walrus build encodes at most one sync-wait on the CTRL-format
    Drain instruction; split the Tile exit drain's waits across single-wait
    sync nops."""
    if getattr(tile.TileContext, '_drain_patched', False):
        return

    def _drain_and_barrier(self, tick_clock, wait_clock):
        from concourse.tile import ScopedClock
        nc = self.nc
        drain_inst = nc.sync.drain()
        wait_clock.add_sem_waits(
            drain_inst.ins, ScopedClock({None: tick_clock.global_clock})
        )
        si = drain_inst.ins.sync_info
        waits = list(si.on_wait) if si and si.on_wait else []
        if len(waits) > 1:
            si.on_wait.clear()
            si.on_wait.append(waits[0])
            for w in waits[1:]:
                nop = nc.sync.nop()
                nsi = nop.ins.sync_info
                if nsi is None:
                    nop.ins.sync_info = mybir.SyncInfo(on_wait=[w], on_update=[])
                else:
                    nsi.on_wait.append(w)
        nc.all_engine_barrier()
        popped = nc._tile_sem_poison_stack.pop()
        assert popped is self._sem_poison
        nc.clear_and_free_semaphores(list(self.sems.allocated().values()))
        nc.all_engine_barrier()

    tile.TileContext._drain_and_barrier = _drain_and_barrier
    tile.TileContext._drain_patched = True


_patch_drain_and_barrier()


def _split_multi_waits(nc):
    """This walrus build encodes only ONE sync-wait per instruction (any
    format).  Hoist every extra wait onto an InstNoOp inserted immediately
    before the instruction on the same engine (per-engine program order
    within a block makes the nops' waits complete first)."""
    for f in nc.m.functions:
        for bb in f.blocks:
            new_insts = []
            for inst in bb.instructions:
                si = inst.sync_info
                waits = list(si.on_wait) if si and si.on_wait else []
                if len(waits) > 1:
                    si.on_wait.clear()
                    for w in waits[:-1]:
                        nop = mybir.InstNoOp(
                            name=nc.get_next_instruction_name(), ins=[], outs=[]
                        )
                        nop.engine = inst.engine
                        nop.sync_info = mybir.SyncInfo(on_wait=[w], on_update=[])
                        nc.register_instruction(nop, overwrite=True)
                        new_insts.append(nop)
                    si.on_wait.append(waits[-1])
                new_insts.append(inst)
            bb.instructions[:] = new_insts


def build_routing(labels, n_cores=N_CORES, ns=NS, p=P):
    """Host-side sharding metadata.  Returns per-core
    (shard_rows, gidx[P, C] int32, slots[P, C] f32) plus cols_per_tile.

    shard_rows: global feature-row indices owned by core k (original order).
    gidx[:, c]: local (within-shard) feature rows feeding column c.
    slots[:, c]: slot (center - tile_base) per row, -1.0 for padding.
    Column -> center-tile map (cols_per_tile) is identical across cores.
    """
    labels = np.asarray(labels).astype(np.int64).ravel()
    t_tiles = (ns + p - 1) // p
    shard_rows = []
    loc_sorted = []   # per core: bucket-local labels, sorted
    lidx_sorted = []  # per core: local shard index per sorted row
    for k in range(n_cores):
        lo = k * ns
        rows = np.nonzero((labels >= lo) & (labels < lo + ns))[0]
        loc = labels[rows] - lo
        order = np.argsort(loc, kind='stable')
        shard_rows.append(rows)
        loc_sorted.append(loc[order])
        lidx_sorted.append(order.astype(np.int64))

    # rows per (core, tile) -> shared column structure
    counts = np.zeros((n_cores, t_tiles), dtype=np.int64)
    for k in range(n_cores):
        tl = loc_sorted[k] // p
        cnt = np.bincount(tl, minlength=t_tiles)
        counts[k] = cnt[:t_tiles]
    cols_per_tile = np.maximum(1, -(-counts.max(axis=0) // p)).astype(int)
    c_total = int(cols_per_tile.sum())

    gidx_all, slots_all = [], []
    for k in range(n_cores):
        gidx = np.zeros((p, c_total), dtype=np.int64)
        slots = np.full((p, c_total), -1.0, dtype=np.float32)
        tl = loc_sorted[k] // p
        starts = np.searchsorted(tl, np.arange(t_tiles))
        ends = np.searchsorted(tl, np.arange(t_tiles), side='right')
        col = 0
        for t in range(t_tiles):
            sel = slice(starts[t], ends[t])
            lidx = lidx_sorted[k][sel]
            slot = (loc_sorted[k][sel] - t * p).astype(np.float32)
            r = len(lidx)
            for j in range(cols_per_tile[t]):
                a, b = j * p, min((j + 1) * p, r)
                if a < b:
                    gidx[: b - a, col] = lidx[a:b]
                    slots[: b - a, col] = slot[a:b]
                col += 1
        # dma_gather index layout: flat position i = col*128 + p (the gather
        # writes dst[i%128, i//128]); int16, wrapped idx[i] -> [i%16, i//16],
        # replicated across all 8 groups of 16 partitions.
        flat = gidx.T.ravel()  # position i = c*128 + p
        assert flat.max(initial=0) < 32768
        n_i = flat.size  # = c_total * 128, multiple of 16
        wrapped = flat.reshape(n_i // 16, 16).T.astype(np.int16)  # [16, n/16]
        gidx_all.append(np.tile(wrapped, (8, 1)))  # [128, n/16]
        slots_all.append(slots)
    return shard_rows, gidx_all, slots_all, cols_per_tile


def build_program(c_total, cols_per_tile, fpad, ns=NS, d=D, chunk_tiles=7,
                  swdge_queues=2, single_packet=True):
    """Build the (SPMD-shared) Bass program."""
    p = P
    fw = d + 64  # feature-shard row width: 256 features + ones col + pad
    t_tiles = len(cols_per_tile)
    nidx_total = c_total * p
    nc = bass.Bass(num_swdge_queues=swdge_queues)
    feats = nc.declare_dram_parameter('feats', [fpad, fw], F32, isOutput=False)
    centers = nc.declare_dram_parameter('centers', [ns, d], F32, isOutput=False)
    gidx_d = nc.declare_dram_parameter('gidx', [p, nidx_total // 16], I16, isOutput=False)
    slots_d = nc.declare_dram_parameter('slots', [p, c_total], F32, isOutput=False)
    iotam_d = nc.declare_dram_parameter('iotam', [p, p], F32, isOutput=False)
    out = nc.declare_dram_parameter('out', [ns, d], F32, isOutput=True)

    W = d + 1  # psum width: 256 featsum cols + 1 count col

    with tile.TileContext(nc) as tc:
        with (
            tc.tile_pool(name='const', bufs=1) as cpool,
            tc.tile_pool(name='gather', bufs=3) as gpool,
            tc.tile_pool(name='cent', bufs=3) as centpool,
            tc.tile_pool(name='outp', bufs=3) as opool,
            tc.tile_pool(name='oh', bufs=6) as ohpool,
            tc.tile_pool(name='scale', bufs=6) as spool,
            tc.tile_pool(name='psum', bufs=6, space='PSUM') as pspool,
        ):
            nc.gpsimd.load_library(library_config.mlp)
            iota_f = cpool.tile([p, p], F32)
            nc.sync.dma_start(out=iota_f[:], in_=iotam_d[:])
            gidx_sb = cpool.tile([p, nidx_total // 16], I16)
            slots_sb = cpool.tile([p, c_total], F32)
            nc.sync.dma_start(out=gidx_sb[:], in_=gidx_d[:])
            nc.sync.dma_start(out=slots_sb[:], in_=slots_d[:])

            col0 = 0
            for ci, cs in enumerate(range(0, t_tiles, chunk_tiles)):
                chunk = list(range(cs, min(cs + chunk_tiles, t_tiles)))
                nct_chunk = len(chunk)
                ncols = int(sum(cols_per_tile[t] for t in chunk))
                nidx = ncols * p
                rows0 = cs * p
                crows = min(ns, (cs + nct_chunk) * p) - rows0
                full = (crows == nct_chunk * p)

                gbuf = gpool.tile([p, ncols * fw], F32, tag='gbuf')
                g3 = gbuf[:].rearrange('p (c w) -> p c w', w=fw)
                nc.gpsimd.dma_gather(
                    out_ap=g3[:, :, :],
                    in_ap=feats[:],
                    idxs_ap=gidx_sb[:, col0 * 8:(col0 + ncols) * 8],
                    num_idxs=nidx,
                    num_idxs_reg=nidx,
                    elem_size=fw,
                    queue_num=ci % swdge_queues,
                    single_packet=single_packet,
                )
                # chunk-batched centers load / output store (per-tile on the
                # final partial chunk)
                cload = centpool.tile([p, nct_chunk * d], F32, tag='cent')
                ostage = opool.tile([p, nct_chunk * d], F32, tag='ostage')
                if full:
                    nc.sync.dma_start(
                        out=cload[:].rearrange('p (t w) -> p t w', w=d),
                        in_=centers[rows0:rows0 + crows, :].rearrange(
                            '(t p) w -> p t w', p=p),
                    )
                cj = 0
                for tloc, t in enumerate(chunk):
                    pt = min(p, ns - t * p)
                    if not full:
                        nc.sync.dma_start(
                            out=cload[:pt, tloc * d:(tloc + 1) * d],
                            in_=centers[t * p:t * p + pt, :])
                    ps = pspool.tile([p, W], F32, tag='ps')
                    nct = int(cols_per_tile[t])
                    for j in range(nct):
                        oh = ohpool.tile([p, p], F32, tag='oh')
                        nc.vector.tensor_scalar(
                            oh[:], iota_f[:],
                            slots_sb[:, col0 + cj:col0 + cj + 1], SCALE,
                            op0=mybir.AluOpType.is_equal,
                            op1=mybir.AluOpType.mult,
                        )
                        nc.tensor.matmul(
                            ps[:], lhsT=oh[:],
                            rhs=gbuf[:, cj * fw:cj * fw + W],
                            start=(j == 0), stop=(j == nct - 1),
                        )
                        cj += 1
                    # scale_vec = 1 - 0.1*count  (psum col d holds 0.1*count)
                    scale = spool.tile([p, 1], F32, tag='scale')
                    nc.scalar.activation(
                        scale[:], ps[:, d:],
                        mybir.ActivationFunctionType.Identity,
                        bias=1.0, scale=-1.0,
                    )
                    # out = centers * scale_vec  (ACT)  + 0.1*featsum  (DVE)
                    osl = ostage[:pt, tloc * d:(tloc + 1) * d]
                    nc.scalar.activation(
                        osl, cload[:pt, tloc * d:(tloc + 1) * d],
                        mybir.ActivationFunctionType.Identity,
                        bias=0.0, scale=scale[:pt, :],
                    )
                    nc.vector.tensor_tensor(
                        osl, osl, ps[:pt, 0:d], op=mybir.AluOpType.add,
                    )
                    if not full:
                        nc.sync.dma_start(
                            out=out[t * p:t * p + pt, :],
                            in_=ostage[:pt, tloc * d:(tloc + 1) * d])
                if full:
                    nc.sync.dma_start(
                        out=out[rows0:rows0 + crows, :].rearrange(
                            '(t p) w -> p t w', p=p),
                        in_=ostage[:].rearrange('p (t w) -> p t w', w=d),
                    )
                col0 += ncols
    _split_multi_waits(nc)
    # encode .instr bytes for extended-ISA instructions (dma_gather,
    # library reload) — bacc normally does this; raw Bass+Tile must not skip
    # it or walrus fails with "ISA wrong length"
    mybir.codegen_inst_isa_subclasses(nc)
    return nc


_PROGRAM_CACHE = {}

# test-harness knobs: when TRACE is set, pass trace=True through to
# run_bass_kernel_spmd and stash the BassKernelResults in LAST_RESULTS.
TRACE = False
TRACE_TMPDIR = None
LAST_RESULTS = None


def _get_program(c_total, cols_key, fpad):
    key = (c_total, cols_key, fpad)
    if key not in _PROGRAM_CACHE:
        _PROGRAM_CACHE[key] = build_program(
            c_total, np.asarray(cols_key, dtype=int), fpad
        )
    return _PROGRAM_CACHE[key]


def kernel(features, labels, centers):
    features = np.ascontiguousarray(np.asarray(features), dtype=np.float32)
    centers_np = np.ascontiguousarray(np.asarray(centers), dtype=np.float32)
    labels_np = np.asarray(labels)

    shard_rows, gidx_all, slots_all, cols_per_tile = build_routing(labels_np)
    c_total = int(cols_per_tile.sum())
    fpad = max(1, max(len(r) for r in shard_rows))

    nc = _get_program(c_total, tuple(int(x) for x in cols_per_tile), fpad)

    in_maps = []
    for k in range(N_CORES):
        fshard = np.zeros((fpad, D + 64), dtype=np.float32)
        rows = shard_rows[k]
        fshard[: len(rows), :D] = features[rows]
        fshard[:, D] = 1.0
        in_maps.append({
            'feats': fshard,
            'centers': centers_np[k * NS:(k + 1) * NS],
            'gidx': gidx_all[k],
            'slots': slots_all[k],
            'iotam': IOTA_MAT,
        })

    kwargs = {}
    if TRACE:
        kwargs['trace'] = True
        if TRACE_TMPDIR:
            kwargs['tmpdir'] = TRACE_TMPDIR
    res = bass_utils.run_bass_kernel_spmd(
        nc, in_maps, core_ids=list(range(N_CORES)), **kwargs
    )
    global LAST_RESULTS
    LAST_RESULTS = res
    out = np.concatenate([res.results[k]['out'] for k in range(N_CORES)], axis=0)
    return out
